# revision 1
# baseline (speedup 1.0000x reference)
"""Trainium2 Bass kernel for nn_EtaWeights: elementwise loss weighting.

reference:  out = where(loss > eta, loss * mask * eta, -loss / eta + 1.0)

Both branches are affine in loss.  With s1 = mask*eta and s2 = -1/eta:
  true  branch: s1 * loss
  false branch: s2 * loss + 1
When s1 == 0 and eta > 0 (the actual module parameters: mask=0, eta=0.5) the
false branch s2*loss + 1 is >= 0 exactly on loss <= eta and < 0 on loss > eta,
so   out == relu(s2 * loss + 1)   — one ScalarEngine ACTIVATE per tile.
The scalars are read from the (host-side) eta/mask input arrays at call time
and baked into the program as immediates; a general DVE path covers other
parameter values.

Sharding: trivially data-parallel — the 2**25-element loss vector is split
contiguously across the 8 NeuronCores; each core streams its 16 MiB shard
through SBUF (DMA in -> ACT relu in-place -> DMA out).  Memory-bound:
~33.5 MB of HBM traffic per core at the ~420 GB/s SBUF-fabric rate gives a
~80 us streaming floor; measured exec ~91.5 us incl. fixed NEFF pre/post-
amble.  The DMA engines are busy wall-to-wall (zero idle) in the profile.

Implementation notes (raw Bacc, no TileContext):
- Loads are issued by SP/sync (qSPDynamicHW HWDGE ring), stores by the
  Scalar/ACT engine (qActDynamicHW ring); the SDMA engines round-robin the
  two rings so the streams share bandwidth and stores trail the relu
  pipeline by ~1 tile.  All-HWDGE beats SWDGE loads by ~0.7 us: declaring
  the SWDGE queue adds fixed queue setup/teardown to the NEFF.
- Phase-separating loads and stores (stores gated on the last load) is
  ~2 us SLOWER — there is no HBM read/write turnaround penalty to recover,
  and the transition exposes the last relu.
- One semaphore per load tile: DMA completion increments are per-SDMA-
  engine (16 per DMA), so a single cumulative counter is only sound when
  waited at its MAXIMUM value; intermediate thresholds can be satisfied
  with a lagging engine still in flight.  (The final store wait IS at the
  max value, so one cumulative store sem is sound there.)
- ACT -> store ordering needs an explicit semaphore even on one engine:
  the sequencer dispatches the DMA trigger while ACTIVATE is still in the
  datapath.
- Bacc (not Bass) is required: its generate_event_semaphores pass splits
  multi-wait instructions; walrus codegen supports only one sync wait per
  instruction and hard-fails otherwise ("Too many sync wait commands").
- The Block-exit all-engine barrier (incl. gpsimd dge_drain) measurably
  HELPS: without it the SWDGE teardown lands mid-stream and slows the
  transfers (A/B: ~92 vs 98-110 us).
"""

import contextlib

import numpy as np

import concourse.bacc as bacc
import concourse.bass as bass
from concourse import mybir
from concourse.bass_utils import run_bass_kernel_spmd

N_CORES = 8
N = 33554432  # 2**25
SHARD = N // N_CORES  # 4194304 = 128 * 32768
P = 128  # SBUF partitions

_program_cache: dict = {}


def _build_fast(s2: float) -> bass.Bass:
    """out = relu(s2 * loss + 1); 8 tiles of [128, 4096] fp32 (2 MiB each)."""
    F = 4096
    nt = SHARD // (P * F)  # 8
    nc = bacc.Bacc(None)
    x = nc.declare_dram_parameter("loss", [SHARD], mybir.dt.float32, isOutput=False)
    y = nc.declare_dram_parameter("out", [SHARD], mybir.dt.float32, isOutput=True)
    xv = x.rearrange("(n p f) -> n p f", p=P, f=F)
    yv = y.rearrange("(n p f) -> n p f", p=P, f=F)

    with contextlib.ExitStack() as ctx:
        buf = ctx.enter_context(nc.sbuf_tensor([P, F * nt], mybir.dt.float32))
        load_sems = [ctx.enter_context(nc.semaphore(f"load{i}")) for i in range(nt)]
        act_sem = ctx.enter_context(nc.semaphore("act_sem"))
        store_sem = ctx.enter_context(nc.semaphore("store_sem"))
        block = ctx.enter_context(nc.Block())

        @block.sync
        def _(sy):
            # even-index loads on the SP HWDGE ring
            for i in range(0, nt, 2):
                sy.dma_start(buf[:, i * F:(i + 1) * F], xv[i]).then_inc(
                    load_sems[i], 16
                )

        @block.scalar
        def _(s):
            # odd-index loads on the ACT HWDGE ring: both rings feed the
            # SDMA engines during the load-only window, and the trigger
            # issue cost (~0.7 us each) is split across two sequencers
            for i in range(1, nt, 2):
                nc.scalar.dma_start(buf[:, i * F:(i + 1) * F], xv[i]).then_inc(
                    load_sems[i], 16
                )
            for i in range(nt):
                s.wait_ge(load_sems[i], 16)
                nc.scalar.activation(
                    buf[:, i * F:(i + 1) * F], buf[:, i * F:(i + 1) * F],
                    mybir.ActivationFunctionType.Relu, bias=1.0, scale=s2,
                ).then_inc(act_sem, 1)
                s.wait_ge(act_sem, i + 1)
                nc.scalar.dma_start(yv[i], buf[:, i * F:(i + 1) * F]).then_inc(
                    store_sem, 16
                )
            s.wait_ge(store_sem, 16 * nt)

    nc.finalize()
    return nc


def _build_general(eta: float, s1: float, s2: float) -> bass.Bass:
    """out = (s2*t + 1) + (t > eta) * ((s1-s2)*t - 1); Tile-scheduled DVE path."""
    import concourse.tile as tile

    F = 8192
    nt = SHARD // (P * F)  # 4
    nc = bacc.Bacc(None)
    x = nc.declare_dram_parameter("loss", [SHARD], mybir.dt.float32, isOutput=False)
    y = nc.declare_dram_parameter("out", [SHARD], mybir.dt.float32, isOutput=True)
    xv = x.rearrange("(n p f) -> n p f", p=P, f=F)
    yv = y.rearrange("(n p f) -> n p f", p=P, f=F)

    with tile.TileContext(nc) as tc:
        with (
            tc.tile_pool(name="tin", bufs=2) as tin,
            tc.tile_pool(name="tyb", bufs=2) as tyb,
            tc.tile_pool(name="twb", bufs=2) as twb,
        ):
            for i in range(nt):
                t = tin.tile([P, F], mybir.dt.float32)
                nc.gpsimd.dma_start(t[:], xv[i])
                yb = tyb.tile([P, F], mybir.dt.float32)
                wb = twb.tile([P, F], mybir.dt.float32)
                nc.vector.tensor_scalar(
                    yb[:], t[:], s2, 1.0,
                    mybir.AluOpType.mult, mybir.AluOpType.add,
                )
                nc.vector.tensor_scalar(
                    wb[:], t[:], s1 - s2, -1.0,
                    mybir.AluOpType.mult, mybir.AluOpType.add,
                )
                # wb *= (t > eta)
                nc.vector.scalar_tensor_tensor(
                    wb[:], t[:], eta, wb[:],
                    mybir.AluOpType.is_gt, mybir.AluOpType.mult,
                )
                nc.vector.tensor_add(t[:], yb[:], wb[:])
                nc.sync.dma_start(yv[i], t[:])
    nc.finalize()
    return nc


def _get_program(eta: float, s1: float, s2: float, fast: bool) -> bass.Bass:
    key = (eta, s1, s2, fast)
    if key not in _program_cache:
        _program_cache[key] = (
            _build_fast(s2) if fast else _build_general(eta, s1, s2)
        )
    return _program_cache[key]


def kernel(loss, eta, mask, _profile=False, **_profile_kwargs):
    loss = np.ascontiguousarray(np.asarray(loss, dtype=np.float32).reshape(-1))
    assert loss.shape == (N,), loss.shape
    eta_f = float(np.asarray(eta).reshape(-1)[0])
    mask_f = float(np.asarray(mask).reshape(-1)[0])

    s1 = np.float32(mask_f) * np.float32(eta_f)  # true-branch slope
    s2 = -(np.float32(1.0) / np.float32(eta_f))  # false-branch slope
    fast = (s1 == 0.0) and (eta_f > 0.0) and np.isfinite(s2)

    nc = _get_program(eta_f, float(s1), float(s2), bool(fast))

    shards = loss.reshape(N_CORES, SHARD)
    in_maps = [{"loss": shards[i]} for i in range(N_CORES)]
    res = run_bass_kernel_spmd(
        nc, in_maps, list(range(N_CORES)), trace=_profile, **_profile_kwargs
    )
    out = np.concatenate([np.asarray(r["out"]).reshape(-1) for r in res.results])
    if _profile:
        return out, res
    return out



# revision 2
# speedup vs baseline: 1.5072x; 1.5072x over previous
"""Trainium2 Bass kernel for nn_EtaWeights: elementwise loss weighting.

reference:  out = where(loss > eta, loss * mask * eta, -loss / eta + 1.0)

Both branches are affine in loss.  With s1 = mask*eta and s2 = -1/eta:
  true  branch: s1 * loss
  false branch: s2 * loss + 1
When s1 == 0 and eta > 0 (the actual module parameters: mask=0, eta=0.5) the
false branch s2*loss + 1 is >= 0 exactly on loss <= eta and < 0 on loss > eta,
so   out == relu(s2 * loss + 1)   — one ScalarEngine ACTIVATE per tile.
The scalars are read from the (host-side) eta/mask input arrays at call time
and baked into the program as immediates; a general DVE path covers other
parameter values.

Sharding: trivially data-parallel — the 2**25-element loss vector is split
contiguously across the 8 NeuronCores; each core streams its 16 MiB shard
through SBUF (DMA in -> ACT relu in-place -> DMA out).

Fast-path schedule (measured ~60.2 us vs ~90.5 us for the naive
load/relu/store pipeline; profile-derived):
- The 16 SDMA engines cap combined HBM<->SBUF traffic at ~424 GB/s/core and
  each HBM stack (~716 GB/s) is shared by 2 cores, so the 33.5 MB of traffic
  cannot stream faster than ~80 us end to end.  But the walrus epilogue
  (full 254-semaphore file clear + double all-engine barrier, ~8 us) and the
  store drain need not serialize: with NO final store-sem wait the Block
  exits right after the last store *trigger*, the epilogue overlaps the
  drain, and the runtime's queue teardown completes the in-flight stores
  (outputs verified bit-exact across dozens of HW reps).
- Store triggers are enqueued DURING the load phase, interleaved with the
  relus.  HWDGE rings are FIFO per engine: each ring's stores sit behind
  that ring's remaining loads, so no store byte moves before the loads are
  done — the load phase runs at the full HBM fair-share rate — yet by the
  time the last load lands, all but the last stores are already enqueued.
  (Bursting all 8 triggers after the relus costs ~1.3 us per back-to-back
  trigger in HWDGE descriptor generation — spreading them hides that.)
- The last pair of tiles is 256 cols (0.125 MiB) so the last relu+trigger
  tail after the final load byte is ~1.5 us instead of ~8 us.
- Loads split across both HWDGE rings (Sync evens, Scalar odds); one
  semaphore per load tile (DMA completion increments are per-SDMA-engine,
  so a cumulative counter is only sound at its maximum value).
- ACT -> store ordering needs an explicit semaphore even on one engine: the
  sequencer dispatches the DMA trigger while ACTIVATE is in the datapath.
- Bacc (not Bass) is required: its generate_event_semaphores pass splits
  multi-wait instructions; walrus codegen supports only one sync wait per
  instruction.  Keep the default Block exit: Block(no_gpsimd_drain=True)
  with in-flight stores wedged the device (NRT_EXEC_UNIT_UNRECOVERABLE).
"""

import contextlib

import numpy as np

import concourse.bacc as bacc
import concourse.bass as bass
from concourse import mybir
from concourse.bass_utils import run_bass_kernel_spmd

N_CORES = 8
N = 33554432  # 2**25
SHARD = N // N_CORES  # 4194304 = 128 * 32768
P = 128  # SBUF partitions

# 6 x 2.625 MiB + 2 x 0.125 MiB; per-ring (even/odd) totals are balanced.
FS = [5376] * 6 + [256, 256]

_program_cache: dict = {}


def _build_fast(s2: float) -> bass.Bass:
    """out = relu(s2 * loss + 1), nowait/spread-trigger schedule (see module
    docstring)."""
    nt = len(FS)
    bounds = [0]
    for F in FS:
        bounds.append(bounds[-1] + F * P)
    assert bounds[-1] == SHARD
    sofs = [0]
    for F in FS:
        sofs.append(sofs[-1] + F)

    nc = bacc.Bacc(None)
    x = nc.declare_dram_parameter("loss", [SHARD], mybir.dt.float32, isOutput=False)
    y = nc.declare_dram_parameter("out", [SHARD], mybir.dt.float32, isOutput=True)

    def xt(i):
        return x[bounds[i]:bounds[i + 1]].rearrange("(p f) -> p f", p=P)

    def yt(i):
        return y[bounds[i]:bounds[i + 1]].rearrange("(p f) -> p f", p=P)

    with contextlib.ExitStack() as ctx:
        buf = ctx.enter_context(nc.sbuf_tensor([P, SHARD // P], mybir.dt.float32))
        load_sems = [ctx.enter_context(nc.semaphore(f"load{i}")) for i in range(nt)]
        act_sem = ctx.enter_context(nc.semaphore("act_sem"))
        store_sem = ctx.enter_context(nc.semaphore("store_sem"))
        block = ctx.enter_context(nc.Block())

        def bt(i):
            return buf[:, sofs[i]:sofs[i + 1]]

        @block.sync
        def _(sy):
            # even loads on the SP HWDGE ring
            for i in range(0, nt, 2):
                sy.dma_start(bt(i), xt(i)).then_inc(load_sems[i], 16)
            # odd stores, each gated on its tile's relu; they enqueue on the
            # SP ring BEHIND the even loads (FIFO), so they cannot drain
            # before this ring's loads finish.
            for i in range(1, nt, 2):
                sy.wait_ge(act_sem, i + 1)
                sy.dma_start(yt(i), bt(i)).then_inc(store_sem, 16)

        @block.scalar
        def _(s):
            # odd loads on the ACT HWDGE ring
            for i in range(1, nt, 2):
                nc.scalar.dma_start(bt(i), xt(i)).then_inc(load_sems[i], 16)
            # r0,r1,S0,r2,r3,S2,...  (even stores trail their relu by one
            # tile; they enqueue behind the odd loads on this ring)
            for i in range(nt):
                s.wait_ge(load_sems[i], 16)
                nc.scalar.activation(
                    bt(i), bt(i),
                    mybir.ActivationFunctionType.Relu, bias=1.0, scale=s2,
                ).then_inc(act_sem, 1)
                if i % 2 == 1:
                    nc.scalar.dma_start(yt(i - 1), bt(i - 1)).then_inc(
                        store_sem, 16
                    )
            # no final store-sem wait: the Block exits once the triggers are
            # enqueued and the walrus epilogue overlaps the store drain.

    nc.finalize()
    return nc


def _build_general(eta: float, s1: float, s2: float) -> bass.Bass:
    """out = (s2*t + 1) + (t > eta) * ((s1-s2)*t - 1); Tile-scheduled DVE path."""
    import concourse.tile as tile

    F = 8192
    nt = SHARD // (P * F)  # 4
    nc = bacc.Bacc(None)
    x = nc.declare_dram_parameter("loss", [SHARD], mybir.dt.float32, isOutput=False)
    y = nc.declare_dram_parameter("out", [SHARD], mybir.dt.float32, isOutput=True)
    xv = x.rearrange("(n p f) -> n p f", p=P, f=F)
    yv = y.rearrange("(n p f) -> n p f", p=P, f=F)

    with tile.TileContext(nc) as tc:
        with (
            tc.tile_pool(name="tin", bufs=2) as tin,
            tc.tile_pool(name="tyb", bufs=2) as tyb,
            tc.tile_pool(name="twb", bufs=2) as twb,
        ):
            for i in range(nt):
                t = tin.tile([P, F], mybir.dt.float32)
                nc.gpsimd.dma_start(t[:], xv[i])
                yb = tyb.tile([P, F], mybir.dt.float32)
                wb = twb.tile([P, F], mybir.dt.float32)
                nc.vector.tensor_scalar(
                    yb[:], t[:], s2, 1.0,
                    mybir.AluOpType.mult, mybir.AluOpType.add,
                )
                nc.vector.tensor_scalar(
                    wb[:], t[:], s1 - s2, -1.0,
                    mybir.AluOpType.mult, mybir.AluOpType.add,
                )
                # wb *= (t > eta)
                nc.vector.scalar_tensor_tensor(
                    wb[:], t[:], eta, wb[:],
                    mybir.AluOpType.is_gt, mybir.AluOpType.mult,
                )
                nc.vector.tensor_add(t[:], yb[:], wb[:])
                nc.sync.dma_start(yv[i], t[:])
    nc.finalize()
    return nc


def _get_program(eta: float, s1: float, s2: float, fast: bool) -> bass.Bass:
    key = (eta, s1, s2, fast)
    if key not in _program_cache:
        _program_cache[key] = (
            _build_fast(s2) if fast else _build_general(eta, s1, s2)
        )
    return _program_cache[key]


def kernel(loss, eta, mask, _profile=False, **_profile_kwargs):
    loss = np.ascontiguousarray(np.asarray(loss, dtype=np.float32).reshape(-1))
    assert loss.shape == (N,), loss.shape
    eta_f = float(np.asarray(eta).reshape(-1)[0])
    mask_f = float(np.asarray(mask).reshape(-1)[0])

    s1 = np.float32(mask_f) * np.float32(eta_f)  # true-branch slope
    s2 = -(np.float32(1.0) / np.float32(eta_f))  # false-branch slope
    fast = (s1 == 0.0) and (eta_f > 0.0) and np.isfinite(s2)

    nc = _get_program(eta_f, float(s1), float(s2), bool(fast))

    shards = loss.reshape(N_CORES, SHARD)
    in_maps = [{"loss": shards[i]} for i in range(N_CORES)]
    res = run_bass_kernel_spmd(
        nc, in_maps, list(range(N_CORES)), trace=_profile, **_profile_kwargs
    )
    out = np.concatenate([np.asarray(r["out"]).reshape(-1) for r in res.results])
    if _profile:
        return out, res
    return out


# revision 3
# speedup vs baseline: 1.5687x; 1.0408x over previous
"""Trainium2 Bass kernel for nn_EtaWeights: elementwise loss weighting.

reference:  out = where(loss > eta, loss * mask * eta, -loss / eta + 1.0)

Both branches are affine in loss.  With s1 = mask*eta and s2 = -1/eta:
  true  branch: s1 * loss
  false branch: s2 * loss + 1
When s1 == 0 and eta > 0 (the actual module parameters: mask=0, eta=0.5) the
false branch s2*loss + 1 is >= 0 exactly on loss <= eta and < 0 on loss > eta,
so   out == relu(s2 * loss + 1)   — one ScalarEngine ACTIVATE per tile.
The scalars are read from the (host-side) eta/mask input arrays at call time
and baked into the program as immediates; a general DVE path covers other
parameter values.

Sharding: trivially data-parallel — the 2**25-element loss vector is split
contiguously across the 8 NeuronCores; each core streams its 16 MiB shard
through SBUF (DMA in -> ACT relu in-place -> DMA out).

Fast-path schedule (measured ~60.2 us vs ~90.5 us for the naive
load/relu/store pipeline; profile-derived):
- The 16 SDMA engines cap combined HBM<->SBUF traffic at ~424 GB/s/core and
  each HBM stack (~716 GB/s) is shared by 2 cores, so the 33.5 MB of traffic
  cannot stream faster than ~80 us end to end.  But the walrus epilogue
  (full 254-semaphore file clear + double all-engine barrier, ~8 us) and the
  store drain need not serialize: with NO final store-sem wait the Block
  exits right after the last store *trigger*, the epilogue overlaps the
  drain, and the runtime's queue teardown completes the in-flight stores
  (outputs verified bit-exact across dozens of HW reps).
- Store triggers are enqueued DURING the load phase, interleaved with the
  relus.  HWDGE rings are FIFO per engine: each ring's stores sit behind
  that ring's remaining loads, so no store byte moves before the loads are
  done — the load phase runs at the full HBM fair-share rate — yet by the
  time the last load lands, all but the last stores are already enqueued.
  (Bursting all 8 triggers after the relus costs ~1.3 us per back-to-back
  trigger in HWDGE descriptor generation — spreading them hides that.)
- The last pair of tiles is 256 cols (0.125 MiB) so the last relu+trigger
  tail after the final load byte is ~1.5 us instead of ~8 us.
- Loads split across both HWDGE rings (Sync evens, Scalar odds); one
  semaphore per load tile (DMA completion increments are per-SDMA-engine,
  so a cumulative counter is only sound at its maximum value).
- ACT -> store ordering needs an explicit semaphore even on one engine: the
  sequencer dispatches the DMA trigger while ACTIVATE is in the datapath.
- Bacc (not Bass) is required: its generate_event_semaphores pass splits
  multi-wait instructions; walrus codegen supports only one sync wait per
  instruction.  Keep the default Block exit: Block(no_gpsimd_drain=True)
  with in-flight stores wedged the device (NRT_EXEC_UNIT_UNRECOVERABLE).
"""

import contextlib

import numpy as np

import concourse.bacc as bacc
import concourse.bass as bass
from concourse import mybir
from concourse.bass_utils import run_bass_kernel_spmd

N_CORES = 8
N = 33554432  # 2**25
SHARD = N // N_CORES  # 4194304 = 128 * 32768
P = 128  # SBUF partitions

# 8 tiles (4 per HWDGE ring) — HARD LIMIT: >8 DMA instructions per ring
# overflows the 64-descriptor SDMA ring slots and stores race ahead of the
# relus (silent corruption; verified empirically at 10 and 12 tiles).
# Descending sizes so each pair's relu fits in the remaining drain time.
FS_RING = [5376, 4864, 4352, 1792]
FS = [f for F in FS_RING for f in (F, F)]

_program_cache: dict = {}


def _build_fast(s2: float) -> bass.Bass:
    """out = relu(s2 * loss + 1); nowait/spread-trigger schedule, relu split
    Scalar(ACT, even tiles)/Vector(DVE 2-pass, odd tiles); see module
    docstring."""
    nt = len(FS)
    bounds = [0]
    for F in FS:
        bounds.append(bounds[-1] + F * P)
    assert bounds[-1] == SHARD
    sofs = [0]
    for F in FS:
        sofs.append(sofs[-1] + F)

    nc = bacc.Bacc(None)
    x = nc.declare_dram_parameter("loss", [SHARD], mybir.dt.float32, isOutput=False)
    y = nc.declare_dram_parameter("out", [SHARD], mybir.dt.float32, isOutput=True)

    def xt(i):
        return x[bounds[i]:bounds[i + 1]].rearrange("(p f) -> p f", p=P)

    def yt(i):
        return y[bounds[i]:bounds[i + 1]].rearrange("(p f) -> p f", p=P)

    with contextlib.ExitStack() as ctx:
        buf = ctx.enter_context(nc.sbuf_tensor([P, SHARD // P], mybir.dt.float32))
        load_sems = [ctx.enter_context(nc.semaphore(f"load{i}")) for i in range(nt)]
        act_sem = ctx.enter_context(nc.semaphore("act_sem"))
        vec_sem = ctx.enter_context(nc.semaphore("vec_sem"))
        store_sem = ctx.enter_context(nc.semaphore("store_sem"))
        block = ctx.enter_context(nc.Block())

        def bt(i):
            return buf[:, sofs[i]:sofs[i + 1]]

        @block.sync
        def _(sy):
            # even loads on the SP HWDGE ring
            for i in range(0, nt, 2):
                sy.dma_start(bt(i), xt(i)).then_inc(load_sems[i], 16)
            # odd stores, each gated on its tile's DVE relu; they enqueue on
            # the SP ring BEHIND the even loads (FIFO), so they cannot drain
            # before this ring's loads finish.
            for i in range(1, nt, 2):
                sy.wait_ge(vec_sem, (i + 1) // 2)
                sy.dma_start(yt(i), bt(i)).then_inc(store_sem, 16)

        @block.scalar
        def _(s):
            # odd loads on the ACT HWDGE ring
            for i in range(1, nt, 2):
                nc.scalar.dma_start(bt(i), xt(i)).then_inc(load_sems[i], 16)
            # ACT relu on even tiles; each even store right after its relu
            for k, i in enumerate(range(0, nt, 2)):
                s.wait_ge(load_sems[i], 16)
                nc.scalar.activation(
                    bt(i), bt(i),
                    mybir.ActivationFunctionType.Relu, bias=1.0, scale=s2,
                ).then_inc(act_sem, 1)
                s.wait_ge(act_sem, k + 1)
                nc.scalar.dma_start(yt(i), bt(i)).then_inc(store_sem, 16)
            # no final store-sem wait: the Block exits once the triggers are
            # enqueued and the walrus epilogue overlaps the store drain.

        @block.vector
        def _(v):
            # DVE 2-pass relu on odd tiles: y = max(s2*x + 1, 0)
            for i in range(1, nt, 2):
                v.wait_ge(load_sems[i], 16)
                nc.vector.tensor_scalar(
                    bt(i), bt(i), s2, 1.0,
                    mybir.AluOpType.mult, mybir.AluOpType.add,
                )
                nc.vector.tensor_scalar_max(bt(i), bt(i), 0.0).then_inc(vec_sem, 1)

    nc.finalize()
    return nc


def _build_general(eta: float, s1: float, s2: float) -> bass.Bass:
    """out = (s2*t + 1) + (t > eta) * ((s1-s2)*t - 1); Tile-scheduled DVE path."""
    import concourse.tile as tile

    F = 8192
    nt = SHARD // (P * F)  # 4
    nc = bacc.Bacc(None)
    x = nc.declare_dram_parameter("loss", [SHARD], mybir.dt.float32, isOutput=False)
    y = nc.declare_dram_parameter("out", [SHARD], mybir.dt.float32, isOutput=True)
    xv = x.rearrange("(n p f) -> n p f", p=P, f=F)
    yv = y.rearrange("(n p f) -> n p f", p=P, f=F)

    with tile.TileContext(nc) as tc:
        with (
            tc.tile_pool(name="tin", bufs=2) as tin,
            tc.tile_pool(name="tyb", bufs=2) as tyb,
            tc.tile_pool(name="twb", bufs=2) as twb,
        ):
            for i in range(nt):
                t = tin.tile([P, F], mybir.dt.float32)
                nc.gpsimd.dma_start(t[:], xv[i])
                yb = tyb.tile([P, F], mybir.dt.float32)
                wb = twb.tile([P, F], mybir.dt.float32)
                nc.vector.tensor_scalar(
                    yb[:], t[:], s2, 1.0,
                    mybir.AluOpType.mult, mybir.AluOpType.add,
                )
                nc.vector.tensor_scalar(
                    wb[:], t[:], s1 - s2, -1.0,
                    mybir.AluOpType.mult, mybir.AluOpType.add,
                )
                # wb *= (t > eta)
                nc.vector.scalar_tensor_tensor(
                    wb[:], t[:], eta, wb[:],
                    mybir.AluOpType.is_gt, mybir.AluOpType.mult,
                )
                nc.vector.tensor_add(t[:], yb[:], wb[:])
                nc.sync.dma_start(yv[i], t[:])
    nc.finalize()
    return nc


def _get_program(eta: float, s1: float, s2: float, fast: bool) -> bass.Bass:
    key = (eta, s1, s2, fast)
    if key not in _program_cache:
        _program_cache[key] = (
            _build_fast(s2) if fast else _build_general(eta, s1, s2)
        )
    return _program_cache[key]


def kernel(loss, eta, mask, _profile=False, **_profile_kwargs):
    loss = np.ascontiguousarray(np.asarray(loss, dtype=np.float32).reshape(-1))
    assert loss.shape == (N,), loss.shape
    eta_f = float(np.asarray(eta).reshape(-1)[0])
    mask_f = float(np.asarray(mask).reshape(-1)[0])

    s1 = np.float32(mask_f) * np.float32(eta_f)  # true-branch slope
    s2 = -(np.float32(1.0) / np.float32(eta_f))  # false-branch slope
    fast = (s1 == 0.0) and (eta_f > 0.0) and np.isfinite(s2)

    nc = _get_program(eta_f, float(s1), float(s2), bool(fast))

    shards = loss.reshape(N_CORES, SHARD)
    in_maps = [{"loss": shards[i]} for i in range(N_CORES)]
    res = run_bass_kernel_spmd(
        nc, in_maps, list(range(N_CORES)), trace=_profile, **_profile_kwargs
    )
    out = np.concatenate([np.asarray(r["out"]).reshape(-1) for r in res.results])
    if _profile:
        return out, res
    return out
